# revision 4
# baseline (speedup 1.0000x reference)
"""Trainium2 Bass kernel for a 2-layer GRU LM (SEQ=64, B=64, H=1024, V=10000).

Strategy: 8-way tensor-parallel over the 3H gate dimension. Each core owns a
128-wide slice of each gate (r/z/n) of both layers plus a 1280-wide slice of
the (padded) vocab projection. Per sequence step, each core computes its gate
shard, updates its 128-column slice of h, and an AllGather of the transposed
h-shards (both layers fused, one collective per tick) rebuilds the full
hidden state on every core for the next step's matmuls. The output projection
(batched over step pairs) and the input-side gemm gx0 = emb[tok] @ W0 run in
the gaps while the collective is in flight. All matmuls are fp32r.
"""

import numpy as np

SEQ, B, H, V, E, NC = 64, 64, 1024, 10000, 1024, 8
G = 128                # per-core width of each gate slice
SH = 3 * G             # per-core gate shard (r|z|n)
VP = 10240             # padded vocab
VS = VP // NC          # per-core vocab shard (1280)
NK = H // 128          # contraction chunks (8)
NT = SEQ * B           # 4096 rows of (step, batch)

_CACHE = {}


def _build():
    import concourse.bacc as bacc
    import concourse.mybir as mybir
    import concourse.tile as tile

    F32 = mybir.dt.float32
    F32R = mybir.dt.float32r
    AF = mybir.ActivationFunctionType

    nc = bacc.Bacc("TRN2", target_bir_lowering=False, debug=False, num_devices=NC)

    # ---- DRAM I/O ----
    xt = nc.dram_tensor("xt", [E, NT], F32R, kind="ExternalInput")
    u0 = nc.dram_tensor("u0", [H, SH], F32R, kind="ExternalInput")
    w1 = nc.dram_tensor("w1", [H, SH], F32R, kind="ExternalInput")
    u1 = nc.dram_tensor("u1", [H, SH], F32R, kind="ExternalInput")
    fcwt = nc.dram_tensor("fcwt", [H, VS], F32R, kind="ExternalInput")
    # bias rows: [1, SH] each; bw0r folded into gx0 gemm, bu0r into gh0,
    # bw1r into gx1, bu1r into gh1, fcbr into fc.
    bw0r = nc.dram_tensor("bw0r", [1, SH], F32R, kind="ExternalInput")
    bu0r = nc.dram_tensor("bu0r", [1, SH], F32R, kind="ExternalInput")
    bw1r = nc.dram_tensor("bw1r", [1, SH], F32R, kind="ExternalInput")
    bu1r = nc.dram_tensor("bu1r", [1, SH], F32R, kind="ExternalInput")
    fcbr = nc.dram_tensor("fcbr", [1, VS], F32R, kind="ExternalInput")
    onesd = nc.dram_tensor("ones", [1, 128], F32R, kind="ExternalInput")
    identd = nc.dram_tensor("ident", [64, 64], F32, kind="ExternalInput")
    htini = nc.dram_tensor("htini", [2 * H, B], F32R, kind="ExternalInput")
    hsini = nc.dram_tensor("hsini", [128, G], F32, kind="ExternalInput")

    lg = nc.dram_tensor("lg", [NT, VS], F32, kind="ExternalOutput")
    htf = nc.dram_tensor("htf", [2 * H, B], F32, kind="ExternalOutput")

    with tile.TileContext(nc) as tc:
        with (
            tc.tile_pool(name="wp", bufs=1) as wp,
            tc.tile_pool(name="xtp", bufs=3) as xtp,
            tc.tile_pool(name="gx0sp", bufs=2) as gx0sp,
            tc.tile_pool(name="gx0tp", bufs=4) as gx0tp,
            tc.tile_pool(name="hp", bufs=1) as hp,
            tc.tile_pool(name="hlp", bufs=2) as hlp,
            tc.tile_pool(name="gtp", bufs=2) as gtp,
            tc.tile_pool(name="stp", bufs=3) as stp,
            tc.tile_pool(name="fcep", bufs=2) as fcep,
            tc.tile_pool(name="psg", bufs=1, space="PSUM") as psg,
            tc.tile_pool(name="pstr", bufs=2, space="PSUM") as pstr,
            tc.tile_pool(name="psbig", bufs=3, space="PSUM") as psbig,
            tc.tile_pool(name="dpa", bufs=4, space="DRAM") as dpa,
            tc.tile_pool(name="dpo", bufs=4, space="DRAM") as dpo,
            tc.tile_pool(name="dgx", bufs=1, space="DRAM") as dgx,
        ):
            # ---- resident SBUF ----
            u0_sb = wp.tile([128, NK * SH], F32R, tag="u0")
            w1_sb = wp.tile([128, NK * SH], F32R, tag="w1")
            u1_sb = wp.tile([128, NK * SH], F32R, tag="u1")
            fcwt_sb = wp.tile([128, NK * VS], F32R, tag="fcwt")
            ones_sb = wp.tile([1, 128], F32R, tag="ones")
            ident_sb = wp.tile([64, 64], F32, tag="ident")
            bu0_sb = wp.tile([1, SH], F32R, tag="bu0")
            bw0_sb = wp.tile([1, SH], F32R, tag="bw0")
            bw1_sb = wp.tile([1, SH], F32R, tag="bw1")
            bu1_sb = wp.tile([1, SH], F32R, tag="bu1")
            fcb_sb = wp.tile([1, VS], F32R, tag="fcb")
            h0t_sb = hp.tile([128, NK * 4 * B], F32R, tag="h0t")
            h1t_sb = hp.tile([128, NK * 4 * B], F32R, tag="h1t")

            for sb, dr in ((u0_sb, u0), (w1_sb, w1), (u1_sb, u1)):
                nc.sync.dma_start(
                    sb[:].rearrange("p (k n) -> p k n", k=NK),
                    dr[:].rearrange("(k p) n -> p k n", p=128),
                )
            nc.sync.dma_start(
                fcwt_sb[:].rearrange("p (k n) -> p k n", k=NK),
                fcwt[:].rearrange("(k p) n -> p k n", p=128),
            )
            nc.sync.dma_start(ones_sb[:], onesd[:])
            nc.sync.dma_start(ident_sb[:], identd[:])
            for sb, dr in (
                (bu0_sb, bu0r), (bw0_sb, bw0r), (bw1_sb, bw1r),
                (bu1_sb, bu1r), (fcb_sb, fcbr),
            ):
                nc.sync.dma_start(sb[:], dr[:])

            h0t = h0t_sb[:].rearrange("p (k s b) -> p k s b", k=NK, s=4)
            h1t = h1t_sb[:].rearrange("p (k s b) -> p k s b", k=NK, s=4)
            u0v = u0_sb[:].rearrange("p (k n) -> p k n", k=NK)
            w1v = w1_sb[:].rearrange("p (k n) -> p k n", k=NK)
            u1v = u1_sb[:].rearrange("p (k n) -> p k n", k=NK)
            fcv = fcwt_sb[:].rearrange("p (k n) -> p k n", k=NK)

            # initial hidden (transposed) into slot 3 ( = step -1 )
            hti = htini[:].rearrange("(l k p) b -> l p k b", l=2, p=128)
            nc.sync.dma_start(h0t[:, :, 3, :], hti[0])
            nc.sync.dma_start(h1t[:, :, 3, :], hti[1])

            h0c = hlp.tile([64, G], F32, tag="h0c")
            h1c = hlp.tile([64, G], F32, tag="h1c")
            nc.sync.dma_start(h0c[:], hsini[0:64, :])
            nc.sync.dma_start(h1c[:], hsini[64:128, :])

            gx0d = dgx.tile([NT, SH], F32)

            # ---------- helpers ----------
            def emit_gx0_mtile(m):
                """gx0 rows [128m, 128m+128) = (steps 2m, 2m+1) @ W0 shard."""
                xt_t = xtp.tile([128, NK * 128], F32R, tag="xt")
                nc.sync.dma_start(
                    xt_t[:].rearrange("p (k n) -> p k n", k=NK),
                    xt[:, m * 128 : (m + 1) * 128].rearrange(
                        "(k p) n -> p k n", p=128
                    ),
                )
                xv = xt_t[:].rearrange("p (k n) -> p k n", k=NK)
                ps = psbig.tile([128, SH], F32, tag="big")
                # weights for layer0 come via u0? no: gx0 uses W0 shard,
                # which the host packs into... (separate tensor) -- w0 below
                for k in range(NK):
                    nc.tensor.matmul(
                        ps[:], xv[:, k, :], w0v[:, k, :],
                        start=(k == 0), stop=False,
                    )
                nc.tensor.matmul(
                    ps[:], ones_sb[:, 0:128], bw0_sb[:],
                    start=False, stop=True,
                )
                ev = gx0sp.tile([128, SH], F32, tag="gx0ev")
                nc.vector.tensor_copy(ev[:], ps[:])
                nc.sync.dma_start(gx0d[m * 128 : (m + 1) * 128, :], ev[:])

            def fetch_gx0(t):
                g = gx0tp.tile([64, SH], F32, tag="gx0t")
                nc.sync.dma_start(g[:], gx0d[t * 64 : (t + 1) * 64, :])
                return g

            def gemm_shard(stat_view, slot, wview, bias_sb, tag):
                """[64, SH] = h^T-slot stationary x weight shard + bias row."""
                ps = psg.tile([64, SH], F32, tag=tag)
                for k in range(NK):
                    nc.tensor.matmul(
                        ps[:], stat_view[:, k, slot, :], wview[:, k, :],
                        start=(k == 0), stop=False,
                    )
                nc.tensor.matmul(
                    ps[:], ones_sb[:, 0:64], bias_sb[:], start=False, stop=True
                )
                return ps

            def gates(gx_rz, gx_n, gh_ps, hc_old, tag):
                """gh_ps: [64, SH] PSUM; gx_*: SBUF slices. Returns h_new."""
                t1 = gtp.tile([64, 256], F32, tag=f"{tag}t1")
                nc.vector.tensor_add(t1[:], gx_rz, gh_ps[:, 0:256])
                rz = gtp.tile([64, 256], F32, tag=f"{tag}rz")
                nc.scalar.activation(rz[:], t1[:], AF.Sigmoid)
                t2 = gtp.tile([64, G], F32, tag=f"{tag}t2")
                nc.vector.tensor_mul(t2[:], rz[:, 0:128], gh_ps[:, 256:384])
                t3 = gtp.tile([64, G], F32, tag=f"{tag}t3")
                nc.vector.tensor_add(t3[:], t2[:], gx_n)
                n = gtp.tile([64, G], F32, tag=f"{tag}n")
                nc.scalar.activation(n[:], t3[:], AF.Tanh)
                d = gtp.tile([64, G], F32, tag=f"{tag}d")
                nc.vector.tensor_sub(d[:], n[:], hc_old[:])
                zd = gtp.tile([64, G], F32, tag=f"{tag}zd")
                nc.vector.tensor_mul(zd[:], rz[:, 128:256], d[:])
                h_new = hlp.tile([64, G], F32, tag=f"{tag}h")
                nc.vector.tensor_add(h_new[:], hc_old[:], zd[:])
                return h_new

            def transpose_to(st, col0, h_new):
                ps = pstr.tile([128, 128], F32, tag="tr")
                nc.tensor.transpose(
                    ps[:, col0 : col0 + 64], h_new[:], ident_sb[:]
                )
                nc.vector.tensor_copy(
                    st[:, col0 : col0 + 64], ps[:, col0 : col0 + 64]
                )

            def emit_fc_pair(p):
                """logits rows [128p, 128p+128) from h1t slots (2p%4, +1)."""
                s0 = (2 * p) % 4
                fce = fcep.tile([128, VS], F32, tag="fce")
                for off, w in ((0, 512), (512, 512), (1024, 256)):
                    ps = psbig.tile([128, 512], F32, tag="big")
                    for k in range(NK):
                        nc.tensor.matmul(
                            ps[:, 0:w],
                            h1t[:, k, s0 : s0 + 2, :],
                            fcv[:, k, off : off + w],
                            start=(k == 0), stop=False,
                        )
                    nc.tensor.matmul(
                        ps[:, 0:w], ones_sb[:], fcb_sb[:, off : off + w],
                        start=False, stop=True,
                    )
                    nc.vector.tensor_copy(fce[:, off : off + w], ps[:, 0:w])
                nc.sync.dma_start(lg[p * 128 : (p + 1) * 128, :], fce[:])

            # w0 shard resident (used only by gx0 gemms)
            w0d = nc.dram_tensor("w0", [E, SH], F32R, kind="ExternalInput")
            w0_sb = wp.tile([128, NK * SH], F32R, tag="w0")
            nc.sync.dma_start(
                w0_sb[:].rearrange("p (k n) -> p k n", k=NK),
                w0d[:].rearrange("(k p) n -> p k n", p=128),
            )
            w0v = w0_sb[:].rearrange("p (k n) -> p k n", k=NK)

            # ---------- prologue ----------
            for m in range(3):
                emit_gx0_mtile(m)
            gx0t_tiles = {0: fetch_gx0(0), 1: fetch_gx0(1)}

            st_prev = None
            # ---------- main loop ----------
            for tau in range(SEQ + 1):
                st = stp.tile([128, 128], F32R, tag="st")
                if tau == 0:
                    nc.vector.memset(st[:, 64:128].bitcast(F32), 0.0)

                # layer-0 step tau
                if tau <= SEQ - 1:
                    gh0 = gemm_shard(h0t, (tau - 1) % 4, u0v, bu0_sb, "gh0")
                # layer-1 step tau-1
                if tau >= 1:
                    gx1 = gemm_shard(h0t, (tau - 1) % 4, w1v, bw1_sb, "gx1")
                    gh1 = gemm_shard(h1t, (tau - 2) % 4, u1v, bu1_sb, "gh1")

                if tau <= SEQ - 1:
                    g0 = gx0t_tiles.pop(tau)
                    h0c_new = gates(
                        g0[:, 0:256], g0[:, 256:384], gh0, h0c, "g0"
                    )
                    transpose_to(st, 0, h0c_new)
                    h0c = h0c_new
                if tau >= 1:
                    gx1sb = gtp.tile([64, SH], F32, tag="gx1sb")
                    nc.scalar.activation(gx1sb[:], gx1[:], AF.Copy)
                    h1c_new = gates(
                        gx1sb[:, 0:256], gx1sb[:, 256:384], gh1, h1c, "g1"
                    )
                    transpose_to(st, 64, h1c_new)
                    h1c = h1c_new

                # ---- fused AllGather of (h0sT[tau], h1sT[tau-1]) ----
                agin = dpa.tile([2 * 128, B], F32R, tag="agin")
                if tau == 0:
                    nc.sync.dma_start(
                        agin[:].rearrange("(q p) b -> p q b", p=128),
                        st[:].rearrange("p (q b) -> p q b", q=2),
                    )
                elif tau == SEQ:
                    nc.sync.dma_start(agin[0:128, :], st_prev[:, 0:64])
                    nc.sync.dma_start(agin[128:256, :], st[:, 64:128])
                else:
                    nc.sync.dma_start(
                        agin[:].rearrange("(q p) b -> p q b", p=128),
                        st[:].rearrange("p (q b) -> p q b", q=2),
                    )
                agout = dpo.tile([NC * 2 * 128, B], F32R, tag="agout")
                nc.gpsimd.collective_compute(
                    "AllGather",
                    mybir.AluOpType.bypass,
                    replica_groups=[list(range(NC))],
                    ins=[agin[:]],
                    outs=[agout[:]],
                )
                agv = agout[:].rearrange("(c q p) b -> q p c b", q=2, p=128)
                if tau <= SEQ - 1:
                    nc.sync.dma_start(h0t[:, :, tau % 4, :], agv[0])
                if tau >= 1:
                    nc.sync.dma_start(h1t[:, :, (tau - 1) % 4, :], agv[1])
                st_prev = st

                # ---- background work ----
                if tau % 2 == 0 and tau // 2 + 3 <= NT // 128 - 1:
                    emit_gx0_mtile(tau // 2 + 3)
                if tau + 2 <= SEQ - 1:
                    gx0t_tiles[tau + 2] = fetch_gx0(tau + 2)
                if tau >= 3 and tau % 2 == 1:
                    emit_fc_pair((tau - 3) // 2)

            # ---------- epilogue ----------
            emit_fc_pair(SEQ // 2 - 1)
            htfv = htf[:].rearrange("(l k p) b -> l p k b", l=2, p=128)
            hout0 = fcep.tile([128, NK * B], F32, tag="hout")
            nc.vector.tensor_copy(
                hout0[:].rearrange("p (k b) -> p k b", k=NK), h0t[:, :, 3, :]
            )
            nc.sync.dma_start(htfv[0], hout0[:].rearrange("p (k b) -> p k b", k=NK))
            hout1 = fcep.tile([128, NK * B], F32, tag="hout")
            nc.vector.tensor_copy(
                hout1[:].rearrange("p (k b) -> p k b", k=NK), h1t[:, :, 3, :]
            )
            nc.sync.dma_start(htfv[1], hout1[:].rearrange("p (k b) -> p k b", k=NK))

    nc.finalize()
    return nc


def _prep_inputs(inputs):
    tok = np.asarray(inputs["inputs"])
    hidden = np.asarray(inputs["hidden"], np.float32)
    emb = np.asarray(inputs["emb"], np.float32)
    W0 = np.asarray(inputs["W0"], np.float32)
    U0 = np.asarray(inputs["U0"], np.float32)
    bw0 = np.asarray(inputs["bw0"], np.float32)
    bu0 = np.asarray(inputs["bu0"], np.float32)
    W1 = np.asarray(inputs["W1"], np.float32)
    U1 = np.asarray(inputs["U1"], np.float32)
    bw1 = np.asarray(inputs["bw1"], np.float32)
    bu1 = np.asarray(inputs["bu1"], np.float32)
    fcW = np.asarray(inputs["fcW"], np.float32)
    fcb = np.asarray(inputs["fcb"], np.float32)

    X = emb[tok.reshape(-1)]                      # [NT, E]
    XT = np.ascontiguousarray(X.T)                # [E, NT]
    fcWT = np.zeros((H, VP), np.float32)
    fcWT[:, :V] = fcW.T
    fcbp = np.zeros((VP,), np.float32)
    fcbp[:V] = fcb

    ones = np.ones((1, 128), np.float32)
    ident = np.eye(64, dtype=np.float32)
    h0T = np.ascontiguousarray(hidden[0].T)       # [H, B]
    h1T = np.ascontiguousarray(hidden[1].T)
    htini = np.concatenate([h0T, h1T], 0)         # [2H, B]

    in_maps = []
    for c in range(NC):
        idx = np.concatenate(
            [np.arange(g * H + c * G, g * H + (c + 1) * G) for g in range(3)]
        )
        hsini = np.concatenate(
            [hidden[0][:, c * G : (c + 1) * G], hidden[1][:, c * G : (c + 1) * G]],
            0,
        ).astype(np.float32)
        in_maps.append(
            {
                "xt": XT,
                "w0": np.ascontiguousarray(W0[:, idx]),
                "u0": np.ascontiguousarray(U0[:, idx]),
                "w1": np.ascontiguousarray(W1[:, idx]),
                "u1": np.ascontiguousarray(U1[:, idx]),
                "fcwt": np.ascontiguousarray(fcWT[:, c * VS : (c + 1) * VS]),
                "bw0r": bw0[idx].reshape(1, SH).copy(),
                "bu0r": bu0[idx].reshape(1, SH).copy(),
                "bw1r": bw1[idx].reshape(1, SH).copy(),
                "bu1r": bu1[idx].reshape(1, SH).copy(),
                "fcbr": fcbp[c * VS : (c + 1) * VS].reshape(1, VS).copy(),
                "ones": ones,
                "ident": ident,
                "htini": htini,
                "hsini": hsini,
            }
        )
    return in_maps


def kernel(**inputs):
    from concourse import bass_utils

    if "nc" not in _CACHE:
        _CACHE["nc"] = _build()
    nc = _CACHE["nc"]

    in_maps = _prep_inputs(inputs)
    r = bass_utils.run_bass_kernel_spmd(
        nc, in_maps, core_ids=list(range(NC)), trace=False
    )
    lgs = [r.results[c]["lg"].reshape(SEQ, B, VS) for c in range(NC)]
    logits = np.concatenate(lgs, axis=2)[:, :, :V]
    htfin = r.results[0]["htf"]                   # [2H, B]
    hidden_f = np.stack(
        [np.ascontiguousarray(htfin[:H].T), np.ascontiguousarray(htfin[H:].T)]
    )
    return logits, hidden_f


# revision 6
# speedup vs baseline: 1.0057x; 1.0057x over previous
"""Trainium2 Bass kernel for a 2-layer GRU LM (SEQ=64, B=64, H=1024, V=10000).

Strategy: 8-way tensor-parallel over the 3H gate dimension. Each core owns a
128-wide slice of each gate (r/z/n) of both layers plus a 1280-wide slice of
the (padded) vocab projection. Per sequence step, each core computes its gate
shard, updates its 128-column slice of h, and an AllGather of the transposed
h-shards (both layers fused, one collective per tick) rebuilds the full
hidden state on every core for the next step's matmuls. The output projection
(batched over step pairs) and the input-side gemm gx0 = emb[tok] @ W0 run in
the gaps while the collective is in flight. All matmuls are fp32r.
"""

import numpy as np

SEQ, B, H, V, E, NC = 64, 64, 1024, 10000, 1024, 8
G = 128                # per-core width of each gate slice
SH = 3 * G             # per-core gate shard (r|z|n)
VP = 10240             # padded vocab
VS = VP // NC          # per-core vocab shard (1280)
NK = H // 128          # contraction chunks (8)
NT = SEQ * B           # 4096 rows of (step, batch)

_CACHE = {}


def _build():
    import concourse.bacc as bacc
    import concourse.mybir as mybir
    import concourse.tile as tile

    F32 = mybir.dt.float32
    F32R = mybir.dt.float32r
    AF = mybir.ActivationFunctionType

    nc = bacc.Bacc("TRN2", target_bir_lowering=False, debug=False, num_devices=NC)

    # ---- DRAM I/O ----
    xt = nc.dram_tensor("xt", [E, NT], F32R, kind="ExternalInput")
    u0 = nc.dram_tensor("u0", [H, SH], F32R, kind="ExternalInput")
    w1 = nc.dram_tensor("w1", [H, SH], F32R, kind="ExternalInput")
    u1 = nc.dram_tensor("u1", [H, SH], F32R, kind="ExternalInput")
    fcwt = nc.dram_tensor("fcwt", [H, VS], F32R, kind="ExternalInput")
    # bias rows: [1, SH] each; bw0r folded into gx0 gemm, bu0r into gh0,
    # bw1r into gx1, bu1r into gh1, fcbr into fc.
    bw0r = nc.dram_tensor("bw0r", [1, SH], F32R, kind="ExternalInput")
    bu0r = nc.dram_tensor("bu0r", [1, SH], F32R, kind="ExternalInput")
    bw1r = nc.dram_tensor("bw1r", [1, SH], F32R, kind="ExternalInput")
    bu1r = nc.dram_tensor("bu1r", [1, SH], F32R, kind="ExternalInput")
    fcbr = nc.dram_tensor("fcbr", [1, VS], F32R, kind="ExternalInput")
    onesd = nc.dram_tensor("ones", [1, 128], F32R, kind="ExternalInput")
    identd = nc.dram_tensor("ident", [64, 64], F32, kind="ExternalInput")
    htini = nc.dram_tensor("htini", [2 * H, B], F32R, kind="ExternalInput")
    hsini = nc.dram_tensor("hsini", [128, G], F32, kind="ExternalInput")

    lg = nc.dram_tensor("lg", [NT, VS], F32, kind="ExternalOutput")
    htf = nc.dram_tensor("htf", [2 * H, B], F32, kind="ExternalOutput")

    with tile.TileContext(nc) as tc:
        with (
            tc.tile_pool(name="wp", bufs=1) as wp,
            tc.tile_pool(name="xtp", bufs=3) as xtp,
            tc.tile_pool(name="gx0sp", bufs=2) as gx0sp,
            tc.tile_pool(name="gx0tp", bufs=4) as gx0tp,
            tc.tile_pool(name="hp", bufs=1) as hp,
            tc.tile_pool(name="hlp", bufs=2) as hlp,
            tc.tile_pool(name="gtp", bufs=2) as gtp,
            tc.tile_pool(name="stp", bufs=3) as stp,
            tc.tile_pool(name="fcep", bufs=2) as fcep,
            tc.tile_pool(name="psg", bufs=1, space="PSUM") as psg,
            tc.tile_pool(name="pstr", bufs=2, space="PSUM") as pstr,
            tc.tile_pool(name="psbig", bufs=3, space="PSUM") as psbig,
            tc.tile_pool(name="dpa", bufs=4, space="DRAM") as dpa,
            tc.tile_pool(name="dpo", bufs=4, space="DRAM") as dpo,
            tc.tile_pool(name="dgx", bufs=1, space="DRAM") as dgx,
        ):
            # ---- resident SBUF ----
            u0_sb = wp.tile([128, NK * SH], F32R, tag="u0")
            w1_sb = wp.tile([128, NK * SH], F32R, tag="w1")
            u1_sb = wp.tile([128, NK * SH], F32R, tag="u1")
            fcwt_sb = wp.tile([128, NK * VS], F32R, tag="fcwt")
            ones_sb = wp.tile([1, 128], F32R, tag="ones")
            ident_sb = wp.tile([64, 64], F32, tag="ident")
            bu0_sb = wp.tile([1, SH], F32R, tag="bu0")
            bw0_sb = wp.tile([1, SH], F32R, tag="bw0")
            bw1_sb = wp.tile([1, SH], F32R, tag="bw1")
            bu1_sb = wp.tile([1, SH], F32R, tag="bu1")
            fcb_sb = wp.tile([1, VS], F32R, tag="fcb")
            h0t_sb = hp.tile([128, NK * 4 * B], F32R, tag="h0t")
            h1t_sb = hp.tile([128, NK * 4 * B], F32R, tag="h1t")

            for sb, dr in ((u0_sb, u0), (w1_sb, w1), (u1_sb, u1)):
                nc.gpsimd.dma_start(
                    sb[:].rearrange("p (k n) -> p k n", k=NK),
                    dr[:].rearrange("(k p) n -> p k n", p=128),
                )
            nc.gpsimd.dma_start(
                fcwt_sb[:].rearrange("p (k n) -> p k n", k=NK),
                fcwt[:].rearrange("(k p) n -> p k n", p=128),
            )
            nc.sync.dma_start(ones_sb[:], onesd[:])
            nc.sync.dma_start(ident_sb[:], identd[:])
            for sb, dr in (
                (bu0_sb, bu0r), (bw0_sb, bw0r), (bw1_sb, bw1r),
                (bu1_sb, bu1r), (fcb_sb, fcbr),
            ):
                nc.sync.dma_start(sb[:], dr[:])

            h0t = h0t_sb[:].rearrange("p (k s b) -> p k s b", k=NK, s=4)
            h1t = h1t_sb[:].rearrange("p (k s b) -> p k s b", k=NK, s=4)
            u0v = u0_sb[:].rearrange("p (k n) -> p k n", k=NK)
            w1v = w1_sb[:].rearrange("p (k n) -> p k n", k=NK)
            u1v = u1_sb[:].rearrange("p (k n) -> p k n", k=NK)
            fcv = fcwt_sb[:].rearrange("p (k n) -> p k n", k=NK)

            # initial hidden (transposed) into slot 3 ( = step -1 )
            hti = htini[:].rearrange("(l k p) b -> l p k b", l=2, p=128)
            nc.sync.dma_start(h0t[:, :, 3, :], hti[0])
            nc.sync.dma_start(h1t[:, :, 3, :], hti[1])

            h0c = hlp.tile([64, G], F32, tag="h0c")
            h1c = hlp.tile([64, G], F32, tag="h1c")
            nc.sync.dma_start(h0c[:], hsini[0:64, :])
            nc.sync.dma_start(h1c[:], hsini[64:128, :])

            gx0d = dgx.tile([NT, SH], F32)

            # ---------- helpers ----------
            def emit_gx0_mtile(m):
                """gx0 rows [128m, 128m+128) = (steps 2m, 2m+1) @ W0 shard."""
                xt_t = xtp.tile([128, NK * 128], F32R, tag="xt")
                nc.gpsimd.dma_start(
                    xt_t[:].rearrange("p (k n) -> p k n", k=NK),
                    xt[:, m * 128 : (m + 1) * 128].rearrange(
                        "(k p) n -> p k n", p=128
                    ),
                )
                xv = xt_t[:].rearrange("p (k n) -> p k n", k=NK)
                ps = psbig.tile([128, SH], F32, tag="big")
                # weights for layer0 come via u0? no: gx0 uses W0 shard,
                # which the host packs into... (separate tensor) -- w0 below
                for k in range(NK):
                    nc.tensor.matmul(
                        ps[:], xv[:, k, :], w0v[:, k, :],
                        start=(k == 0), stop=False,
                    )
                nc.tensor.matmul(
                    ps[:], ones_sb[:, 0:128], bw0_sb[:],
                    start=False, stop=True,
                )
                ev = gx0sp.tile([128, SH], F32, tag="gx0ev")
                nc.vector.tensor_copy(ev[:], ps[:])
                nc.gpsimd.dma_start(gx0d[m * 128 : (m + 1) * 128, :], ev[:])

            def fetch_gx0(t):
                g = gx0tp.tile([64, SH], F32, tag="gx0t")
                nc.gpsimd.dma_start(g[:], gx0d[t * 64 : (t + 1) * 64, :])
                return g

            def gemm_shard(stat_view, slot, wview, bias_sb, tag):
                """[64, SH] = h^T-slot stationary x weight shard + bias row."""
                ps = psg.tile([64, SH], F32, tag=tag)
                for k in range(NK):
                    nc.tensor.matmul(
                        ps[:], stat_view[:, k, slot, :], wview[:, k, :],
                        start=(k == 0), stop=False,
                    )
                nc.tensor.matmul(
                    ps[:], ones_sb[:, 0:64], bias_sb[:], start=False, stop=True
                )
                return ps

            def gates(gx_rz, gx_n, gh_ps, hc_old, tag):
                """gh_ps: [64, SH] PSUM; gx_*: SBUF slices. Returns h_new."""
                t1 = gtp.tile([64, 256], F32, tag=f"{tag}t1")
                nc.vector.tensor_add(t1[:], gx_rz, gh_ps[:, 0:256])
                rz = gtp.tile([64, 256], F32, tag=f"{tag}rz")
                nc.scalar.activation(rz[:], t1[:], AF.Sigmoid)
                t2 = gtp.tile([64, G], F32, tag=f"{tag}t2")
                nc.vector.tensor_mul(t2[:], rz[:, 0:128], gh_ps[:, 256:384])
                t3 = gtp.tile([64, G], F32, tag=f"{tag}t3")
                nc.vector.tensor_add(t3[:], t2[:], gx_n)
                n = gtp.tile([64, G], F32, tag=f"{tag}n")
                nc.scalar.activation(n[:], t3[:], AF.Tanh)
                d = gtp.tile([64, G], F32, tag=f"{tag}d")
                nc.vector.tensor_sub(d[:], n[:], hc_old[:])
                zd = gtp.tile([64, G], F32, tag=f"{tag}zd")
                nc.vector.tensor_mul(zd[:], rz[:, 128:256], d[:])
                h_new = hlp.tile([64, G], F32, tag=f"{tag}h")
                nc.vector.tensor_add(h_new[:], hc_old[:], zd[:])
                return h_new

            def transpose_to(st, col0, h_new):
                ps = pstr.tile([128, 128], F32, tag="tr")
                nc.tensor.transpose(
                    ps[:, col0 : col0 + 64], h_new[:], ident_sb[:]
                )
                nc.vector.tensor_copy(
                    st[:, col0 : col0 + 64], ps[:, col0 : col0 + 64]
                )

            def emit_fc_pair(p):
                """logits rows [128p, 128p+128) from h1t slots (2p%4, +1)."""
                s0 = (2 * p) % 4
                fce = fcep.tile([128, VS], F32, tag="fce")
                for off, w in ((0, 512), (512, 512), (1024, 256)):
                    ps = psbig.tile([128, 512], F32, tag="big")
                    for k in range(NK):
                        nc.tensor.matmul(
                            ps[:, 0:w],
                            h1t[:, k, s0 : s0 + 2, :],
                            fcv[:, k, off : off + w],
                            start=(k == 0), stop=False,
                        )
                    nc.tensor.matmul(
                        ps[:, 0:w], ones_sb[:], fcb_sb[:, off : off + w],
                        start=False, stop=True,
                    )
                    nc.vector.tensor_copy(fce[:, off : off + w], ps[:, 0:w])
                nc.gpsimd.dma_start(lg[p * 128 : (p + 1) * 128, :], fce[:])

            # w0 shard resident (used only by gx0 gemms)
            w0d = nc.dram_tensor("w0", [E, SH], F32R, kind="ExternalInput")
            w0_sb = wp.tile([128, NK * SH], F32R, tag="w0")
            nc.gpsimd.dma_start(
                w0_sb[:].rearrange("p (k n) -> p k n", k=NK),
                w0d[:].rearrange("(k p) n -> p k n", p=128),
            )
            w0v = w0_sb[:].rearrange("p (k n) -> p k n", k=NK)

            # ---------- prologue ----------
            for m in range(3):
                emit_gx0_mtile(m)
            gx0t_tiles = {0: fetch_gx0(0), 1: fetch_gx0(1)}

            st_prev = None
            # ---------- main loop ----------
            for tau in range(SEQ + 1):
                st = stp.tile([128, 128], F32R, tag="st")
                if tau == 0:
                    nc.vector.memset(st[:, 64:128].bitcast(F32), 0.0)

                # layer-0 step tau
                if tau <= SEQ - 1:
                    gh0 = gemm_shard(h0t, (tau - 1) % 4, u0v, bu0_sb, "gh0")
                # layer-1 step tau-1
                if tau >= 1:
                    gx1 = gemm_shard(h0t, (tau - 1) % 4, w1v, bw1_sb, "gx1")
                    gh1 = gemm_shard(h1t, (tau - 2) % 4, u1v, bu1_sb, "gh1")

                if tau <= SEQ - 1:
                    g0 = gx0t_tiles.pop(tau)
                    h0c_new = gates(
                        g0[:, 0:256], g0[:, 256:384], gh0, h0c, "g0"
                    )
                    transpose_to(st, 0, h0c_new)
                    h0c = h0c_new
                if tau >= 1:
                    gx1sb = gtp.tile([64, SH], F32, tag="gx1sb")
                    nc.scalar.activation(gx1sb[:], gx1[:], AF.Copy)
                    h1c_new = gates(
                        gx1sb[:, 0:256], gx1sb[:, 256:384], gh1, h1c, "g1"
                    )
                    transpose_to(st, 64, h1c_new)
                    h1c = h1c_new

                # ---- fused AllGather of (h0sT[tau], h1sT[tau-1]) ----
                agin = dpa.tile([2 * 128, B], F32R, tag="agin")
                if tau == 0:
                    nc.sync.dma_start(
                        agin[:].rearrange("(q p) b -> p q b", p=128),
                        st[:].rearrange("p (q b) -> p q b", q=2),
                    )
                elif tau == SEQ:
                    nc.sync.dma_start(agin[0:128, :], st_prev[:, 0:64])
                    nc.sync.dma_start(agin[128:256, :], st[:, 64:128])
                else:
                    nc.sync.dma_start(
                        agin[:].rearrange("(q p) b -> p q b", p=128),
                        st[:].rearrange("p (q b) -> p q b", q=2),
                    )
                agout = dpo.tile([NC * 2 * 128, B], F32R, tag="agout")
                nc.gpsimd.collective_compute(
                    "AllGather",
                    mybir.AluOpType.bypass,
                    replica_groups=[list(range(NC))],
                    ins=[agin[:]],
                    outs=[agout[:]],
                )
                agv = agout[:].rearrange("(c q p) b -> q p c b", q=2, p=128)
                if tau <= SEQ - 1:
                    nc.sync.dma_start(h0t[:, :, tau % 4, :], agv[0])
                if tau >= 1:
                    nc.sync.dma_start(h1t[:, :, (tau - 1) % 4, :], agv[1])
                st_prev = st

                # ---- background work ----
                if tau % 2 == 0 and tau // 2 + 3 <= NT // 128 - 1:
                    emit_gx0_mtile(tau // 2 + 3)
                if tau + 2 <= SEQ - 1:
                    gx0t_tiles[tau + 2] = fetch_gx0(tau + 2)
                if tau >= 3 and tau % 2 == 1:
                    emit_fc_pair((tau - 3) // 2)

            # ---------- epilogue ----------
            emit_fc_pair(SEQ // 2 - 1)
            htfv = htf[:].rearrange("(l k p) b -> l p k b", l=2, p=128)
            hout0 = fcep.tile([128, NK * B], F32, tag="hout")
            nc.vector.tensor_copy(
                hout0[:].rearrange("p (k b) -> p k b", k=NK), h0t[:, :, 3, :]
            )
            nc.sync.dma_start(htfv[0], hout0[:].rearrange("p (k b) -> p k b", k=NK))
            hout1 = fcep.tile([128, NK * B], F32, tag="hout")
            nc.vector.tensor_copy(
                hout1[:].rearrange("p (k b) -> p k b", k=NK), h1t[:, :, 3, :]
            )
            nc.sync.dma_start(htfv[1], hout1[:].rearrange("p (k b) -> p k b", k=NK))

    nc.finalize()
    return nc


def _prep_inputs(inputs):
    tok = np.asarray(inputs["inputs"])
    hidden = np.asarray(inputs["hidden"], np.float32)
    emb = np.asarray(inputs["emb"], np.float32)
    W0 = np.asarray(inputs["W0"], np.float32)
    U0 = np.asarray(inputs["U0"], np.float32)
    bw0 = np.asarray(inputs["bw0"], np.float32)
    bu0 = np.asarray(inputs["bu0"], np.float32)
    W1 = np.asarray(inputs["W1"], np.float32)
    U1 = np.asarray(inputs["U1"], np.float32)
    bw1 = np.asarray(inputs["bw1"], np.float32)
    bu1 = np.asarray(inputs["bu1"], np.float32)
    fcW = np.asarray(inputs["fcW"], np.float32)
    fcb = np.asarray(inputs["fcb"], np.float32)

    X = emb[tok.reshape(-1)]                      # [NT, E]
    XT = np.ascontiguousarray(X.T)                # [E, NT]
    fcWT = np.zeros((H, VP), np.float32)
    fcWT[:, :V] = fcW.T
    fcbp = np.zeros((VP,), np.float32)
    fcbp[:V] = fcb

    ones = np.ones((1, 128), np.float32)
    ident = np.eye(64, dtype=np.float32)
    h0T = np.ascontiguousarray(hidden[0].T)       # [H, B]
    h1T = np.ascontiguousarray(hidden[1].T)
    htini = np.concatenate([h0T, h1T], 0)         # [2H, B]

    in_maps = []
    for c in range(NC):
        idx = np.concatenate(
            [np.arange(g * H + c * G, g * H + (c + 1) * G) for g in range(3)]
        )
        hsini = np.concatenate(
            [hidden[0][:, c * G : (c + 1) * G], hidden[1][:, c * G : (c + 1) * G]],
            0,
        ).astype(np.float32)
        in_maps.append(
            {
                "xt": XT,
                "w0": np.ascontiguousarray(W0[:, idx]),
                "u0": np.ascontiguousarray(U0[:, idx]),
                "w1": np.ascontiguousarray(W1[:, idx]),
                "u1": np.ascontiguousarray(U1[:, idx]),
                "fcwt": np.ascontiguousarray(fcWT[:, c * VS : (c + 1) * VS]),
                "bw0r": bw0[idx].reshape(1, SH).copy(),
                "bu0r": bu0[idx].reshape(1, SH).copy(),
                "bw1r": bw1[idx].reshape(1, SH).copy(),
                "bu1r": bu1[idx].reshape(1, SH).copy(),
                "fcbr": fcbp[c * VS : (c + 1) * VS].reshape(1, VS).copy(),
                "ones": ones,
                "ident": ident,
                "htini": htini,
                "hsini": hsini,
            }
        )
    return in_maps


def kernel(**inputs):
    from concourse import bass_utils

    if "nc" not in _CACHE:
        _CACHE["nc"] = _build()
    nc = _CACHE["nc"]

    in_maps = _prep_inputs(inputs)
    r = bass_utils.run_bass_kernel_spmd(
        nc, in_maps, core_ids=list(range(NC)), trace=False
    )
    lgs = [r.results[c]["lg"].reshape(SEQ, B, VS) for c in range(NC)]
    logits = np.concatenate(lgs, axis=2)[:, :, :V]
    htfin = r.results[0]["htf"]                   # [2H, B]
    hidden_f = np.stack(
        [np.ascontiguousarray(htfin[:H].T), np.ascontiguousarray(htfin[H:].T)]
    )
    return logits, hidden_f
